# revision 16
# baseline (speedup 1.0000x reference)
"""Distributed Bass kernel for nn_Attention_12635793785074 (8 TRN2 NeuronCores).

Sharding: sequence-parallel. Core i owns query tokens [i*SL, (i+1)*SL).
Each core computes Q/K/V projections for its own tokens (all heads),
AllGathers the (normalized, roped, bf16) K^T and V across the 8 cores,
runs full attention for its queries over all 4096 keys, and applies the
output projection for its rows. The full output is the concatenation of
per-core outputs (no output collective needed).
"""

import os
import sys

for _p in (
    "/opt/trn_rl_repo",
    "/opt/pypackages",
    "/root/.axon_site/_ro/trn_rl_repo",
    "/root/.axon_site/_ro/pypackages",
):
    if os.path.isdir(_p) and _p not in sys.path:
        sys.path.append(_p)

import numpy as np  # noqa: E402

import concourse.bass as bass  # noqa: E402
import concourse.mybir as mybir  # noqa: E402
import concourse.tile as tile  # noqa: E402
from concourse import bacc  # noqa: E402
from concourse.bass_utils import run_bass_kernel_spmd  # noqa: E402

F32 = mybir.dt.float32
F32R = mybir.dt.float32r
BF16 = mybir.dt.bfloat16

B, S, D = 1, 4096, 2048
H, KVH, HD = 16, 4, 128
G = H // KVH  # 4 query heads per kv head
NCORES = 8
EPS = 1e-6
ISQ = 1.0 / float(np.sqrt(HD))  # attention scale, folded into exp()


def _rotmat_np():
    """P^T for rot(v) = P @ v with rot(v)[d] = -v[d+64] (d<64), v[d-64] (d>=64)."""
    P = np.zeros((HD, HD), dtype=np.float32)
    for dd in range(64):
        P[dd, dd + 64] = -1.0
        P[dd + 64, dd] = 1.0
    return P.T.copy()


def _emit(tc, nc, prm, sl):
    """Emit the per-core program. sl = local sequence length (tokens per core)."""
    from contextlib import ExitStack

    nt = sl // 128          # local token tiles
    ltn = (sl * NCORES) // 128  # total l (key) tiles
    ftn = D // 128          # feature tiles of the contraction dim

    x_ap = prm["x"].ap()
    wq_ap = prm["wq"].ap()
    wk_ap = prm["wk"].ap()
    wv_ap = prm["wv"].ap()
    wo_ap = prm["wo"].ap()
    out_ap = prm["out"].ap()
    kv_in = prm["kv_in"]
    kv_out = prm["kv_out"]
    kvper = KVH * 128 * sl + nt * 128 * (KVH * HD)  # elements per core in kv bounce

    def kv_in_k(kh):
        off = kh * 128 * sl
        return kv_in.ap()[off : off + 128 * sl].rearrange("(p t) -> p t", t=sl)

    def kv_in_v(tt):
        off = KVH * 128 * sl + tt * 128 * (KVH * HD)
        return kv_in.ap()[off : off + 128 * (KVH * HD)].rearrange(
            "(p c) -> p c", c=KVH * HD
        )

    def kv_out_k(s_, kh):
        off = kvper * s_ + kh * 128 * sl
        return kv_out.ap()[off : off + 128 * sl].rearrange("(p t) -> p t", t=sl)

    def kv_out_v(s_, tt):
        off = kvper * s_ + KVH * 128 * sl + tt * 128 * (KVH * HD)
        return kv_out.ap()[off : off + 128 * (KVH * HD)].rearrange(
            "(p c) -> p c", c=KVH * HD
        )

    es = ExitStack()
    with es:
        # ---------------- constants ----------------
        consts = es.enter_context(tc.tile_pool(name="consts", bufs=1))
        rotmat_sb = consts.tile([HD, HD], BF16)
        nc.sync.dma_start(out=rotmat_sb, in_=prm["rotmat"].ap())
        ones_f = consts.tile([128, 1], F32)
        nc.vector.memset(ones_f, 1.0)
        ones_sb = consts.tile([128, 1], F32R)
        nc.vector.tensor_copy(ones_sb, ones_f)
        costwq_sb = consts.tile([128, sl], F32)
        sintwq_sb = consts.tile([128, sl], F32)
        costwk_sb = consts.tile([128, sl], F32)
        sintwk_sb = consts.tile([128, sl], F32)
        nc.sync.dma_start(out=costwq_sb, in_=prm["costw_q"].ap())
        nc.sync.dma_start(out=sintwq_sb, in_=prm["sintw_q"].ap())
        nc.sync.dma_start(out=costwk_sb, in_=prm["costw_k"].ap())
        nc.sync.dma_start(out=sintwk_sb, in_=prm["sintw_k"].ap())
        id_bf = consts.tile([128, 128], BF16)
        from concourse.masks import make_identity

        make_identity(nc, id_bf)
        zbias = consts.tile([128, 1], F32)
        nc.vector.memset(zbias, 0.0)
        ebias = consts.tile([128, 1], F32)
        nc.vector.memset(ebias, EPS)

        # persistent big tensors
        bigs = es.enter_context(tc.tile_pool(name="bigs", bufs=1))
        xT = bigs.tile([128, ftn * sl], BF16, name="xT")        # x^T, [feat, tok]
        qt = bigs.tile([128, H * sl], BF16, name="qt")          # roped Q^T per head
        kt = bigs.tile([128, KVH * ltn * 128], BF16, name="kt")  # gathered K^T
        vsb = bigs.tile([128, ltn * (KVH * HD)], BF16, name="vsb")  # gathered V
        otn = bigs.tile([128, H * sl], BF16, name="otn")        # normalized attn out^T

        # ---------------- phase 1: x -> x^T (bf16) ----------------
        with tc.tile_pool(name="xload", bufs=nt) as xload, tc.tile_pool(
            name="ps_x", bufs=2, space="PSUM"
        ) as ps_x:
            xbf = []
            for tt in range(nt):
                xb = xload.tile([128, D], BF16, name=f"xb{tt}", tag="xb")
                nc.gpsimd.dma_start(out=xb, in_=x_ap[128 * tt : 128 * (tt + 1), :])
                xbf.append(xb)
            for ff in range(ftn):
                pst = ps_x.tile([128, nt * 128], BF16, name="pst", tag="pst")
                for tt in range(nt):
                    nc.tensor.transpose(
                        pst[:, 128 * tt : 128 * (tt + 1)],
                        xbf[tt][:, 128 * ff : 128 * (ff + 1)],
                        id_bf,
                    )
                nc.vector.tensor_copy(xT[:, sl * ff : sl * ff + nt * 128], pst)

        # shared pools for projection-phase norm/rope pipeline
        rbp = es.enter_context(tc.tile_pool(name="rbp", bufs=3))
        stp = es.enter_context(tc.tile_pool(name="stp", bufs=4))
        ps_small = es.enter_context(tc.tile_pool(name="ps_small", bufs=2, space="PSUM"))
        proj_es = ExitStack()
        pp = proj_es.enter_context(tc.tile_pool(name="prep", bufs=3))
        ps_rot = proj_es.enter_context(tc.tile_pool(name="ps_rot", bufs=2, space="PSUM"))

        def norm_rope(psrc, cosw, sinw, dst_ap, hname):
            """RMS-norm + RoPE of one head's [HD, sl] transposed proj output."""
            raw = pp.tile([128, sl], F32, name=f"raw{hname}", tag="raw")
            nc.scalar.copy(raw, psrc)  # PSUM -> SBUF
            sq = pp.tile([128, sl], F32R, name=f"sq{hname}", tag="sq")
            nc.vector.tensor_mul(sq, raw, raw)
            ssq = ps_small.tile([1, sl], F32, name=f"ssq{hname}", tag="ssq")
            nc.tensor.matmul(ssq, ones_sb, sq, start=True, stop=True)
            sqs = stp.tile([1, sl], F32, name=f"sqs{hname}", tag="sqs")
            nc.scalar.activation(
                sqs,
                ssq,
                mybir.ActivationFunctionType.Sqrt,
                bias=ebias[0:1, :],
                scale=1.0 / HD,
            )
            rsq = stp.tile([1, sl], F32, name=f"rsq{hname}", tag="rsq")
            nc.vector.reciprocal(rsq, sqs)
            rsqb = rbp.tile([128, sl], F32, name=f"rsqb{hname}", tag="rsqb")
            nc.gpsimd.partition_broadcast(rsqb, rsq)
            qn = pp.tile([128, sl], BF16, name=f"qn{hname}", tag="qn")
            nc.vector.tensor_mul(qn, raw, rsqb)
            rot = ps_rot.tile([128, sl], F32, name=f"rot{hname}", tag="rot")
            nc.tensor.matmul(rot, rotmat_sb, qn, start=True, stop=True)
            t1 = pp.tile([128, sl], F32, name=f"t1{hname}", tag="t1")
            nc.vector.tensor_mul(t1, qn, cosw)
            t2 = pp.tile([128, sl], F32, name=f"t2{hname}", tag="t2")
            nc.vector.tensor_mul(t2, rot, sinw)
            nc.vector.tensor_add(dst_ap, t1, t2)

        # ---------------- phase 2: K^T, V, collective ----------------
        klocal = proj_es.enter_context(tc.tile_pool(name="klocal", bufs=KVH + nt))
        with tc.tile_pool(name="wkv", bufs=3) as wkv, tc.tile_pool(
            name="ps_proj", bufs=4, space="PSUM"
        ) as ps_proj:
            psk = [
                ps_proj.tile([128, sl], F32, name=f"psk{kh}", tag="projps")
                for kh in range(KVH)
            ]
            for ff in range(ftn):
                wk_t = wkv.tile([128, KVH * HD], BF16, name=f"wk{ff}", tag="wk")
                nc.gpsimd.dma_start(out=wk_t, in_=wk_ap[128 * ff : 128 * (ff + 1), :])
                for kh in range(KVH):
                    nc.tensor.matmul(
                        psk[kh],
                        wk_t[:, 128 * kh : 128 * (kh + 1)],
                        xT[:, sl * ff : sl * (ff + 1)],
                        start=(ff == 0),
                        stop=(ff == ftn - 1),
                    )
            for kh in range(KVH):
                kf = klocal.tile([128, sl], BF16, name=f"kf{kh}", tag="kf")
                norm_rope(psk[kh], costwk_sb, sintwk_sb, kf, f"k{kh}")
                nc.sync.dma_start(out=kv_in_k(kh), in_=kf)

            psv = [
                ps_proj.tile([128, KVH * HD], F32, name=f"psv{tt}", tag="projps")
                for tt in range(nt)
            ]
            for ff in range(ftn):
                wv_t = wkv.tile([128, KVH * HD], BF16, name=f"wv{ff}", tag="wk")
                nc.gpsimd.dma_start(out=wv_t, in_=wv_ap[128 * ff : 128 * (ff + 1), :])
                for tt in range(nt):
                    nc.tensor.matmul(
                        psv[tt],
                        xT[:, sl * ff + 128 * tt : sl * ff + 128 * (tt + 1)],
                        wv_t,
                        start=(ff == 0),
                        stop=(ff == ftn - 1),
                    )
            for tt in range(nt):
                vf = klocal.tile([128, KVH * HD], BF16, name=f"vf{tt}", tag="kf")
                nc.vector.tensor_copy(vf, psv[tt])
                nc.sync.dma_start(out=kv_in_v(tt), in_=vf)

        nc.gpsimd.collective_compute(
            "AllGather",
            mybir.AluOpType.bypass,
            replica_groups=[list(range(NCORES))],
            ins=[kv_in.ap().opt()],
            outs=[kv_out.ap().opt()],
        )
        # scatter gathered kv into SBUF layouts
        for kh in range(KVH):
            for s_ in range(NCORES):
                nc.sync.dma_start(
                    out=kt[:, ltn * 128 * kh + sl * s_ : ltn * 128 * kh + sl * s_ + sl],
                    in_=kv_out_k(s_, kh),
                )
        for s_ in range(NCORES):
            for tt in range(nt):
                c0 = (KVH * HD) * (nt * s_ + tt)
                nc.sync.dma_start(
                    out=vsb[:, c0 : c0 + KVH * HD],
                    in_=kv_out_v(s_, tt),
                )

        # ---------------- phase 3: Q ----------------
        with tc.tile_pool(name="wq", bufs=3) as wqp, tc.tile_pool(
            name="ps_projq", bufs=4, space="PSUM"
        ) as ps_projq:
            for hg in range(H // 4):
                psq = [
                    ps_projq.tile([128, sl], F32, name=f"psq{hg}_{hh}", tag="projqps")
                    for hh in range(4)
                ]
                for ff in range(ftn):
                    wq_t = wqp.tile([128, 512], BF16, name=f"wq{hg}_{ff}", tag="wq")
                    nc.gpsimd.dma_start(
                        out=wq_t,
                        in_=wq_ap[128 * ff : 128 * (ff + 1), 512 * hg : 512 * (hg + 1)],
                    )
                    for hh in range(4):
                        nc.tensor.matmul(
                            psq[hh],
                            wq_t[:, 128 * hh : 128 * (hh + 1)],
                            xT[:, sl * ff : sl * (ff + 1)],
                            start=(ff == 0),
                            stop=(ff == ftn - 1),
                        )
                for hh in range(4):
                    h = 4 * hg + hh
                    norm_rope(
                        psq[hh],
                        costwq_sb,
                        sintwq_sb,
                        qt[:, sl * h : sl * (h + 1)],
                        f"q{h}",
                    )

        proj_es.close()  # release proj-phase PSUM banks before attention

        # ---------------- phase 4: attention ----------------
        with tc.tile_pool(name="ps_sc", bufs=2, space="PSUM") as ps_sc, tc.tile_pool(
            name="ps_out", bufs=2, space="PSUM"
        ) as ps_out, tc.tile_pool(name="prpool", bufs=4) as prpool, tc.tile_pool(
            name="pairp", bufs=6
        ) as pairp, tc.tile_pool(name="accp", bufs=2) as accp:
            npair = ltn // 2
            for h in range(H):
                kh = h // G
                qh = qt[:, sl * h : sl * (h + 1)]
                outp = ps_out.tile([128, sl], F32, name=f"outp{h}", tag="outp")
                acc = accp.tile([128, sl], F32R, name=f"acc{h}", tag="acc")
                pairs = []
                nq2 = 0
                for jp in range(npair):
                    j0, j1 = 2 * jp, 2 * jp + 1
                    sc = ps_sc.tile([128, 2 * sl], F32, name=f"sc{h}_{jp}", tag="sc")
                    nc.tensor.matmul(
                        sc[:, 0:sl],
                        kt[:, ltn * 128 * kh + 128 * j0 : ltn * 128 * kh + 128 * (j0 + 1)],
                        qh,
                        start=True,
                        stop=True,
                    )
                    nc.tensor.matmul(
                        sc[:, sl : 2 * sl],
                        kt[:, ltn * 128 * kh + 128 * j1 : ltn * 128 * kh + 128 * (j1 + 1)],
                        qh,
                        start=True,
                        stop=True,
                    )
                    pr = prpool.tile([128, 2 * sl], BF16, name=f"pr{h}_{jp}", tag="pr")
                    nc.scalar.activation(
                        pr,
                        sc,
                        mybir.ActivationFunctionType.Exp,
                        bias=zbias[:, :],
                        scale=ISQ,
                    )
                    # PV accumulate
                    nc.tensor.matmul(
                        outp,
                        vsb[:, (KVH * HD) * j0 + HD * kh : (KVH * HD) * j0 + HD * (kh + 1)],
                        pr[:, 0:sl],
                        start=(jp == 0),
                        stop=False,
                        skip_group_check=True,
                    )
                    nc.tensor.matmul(
                        outp,
                        vsb[:, (KVH * HD) * j1 + HD * kh : (KVH * HD) * j1 + HD * (kh + 1)],
                        pr[:, sl : 2 * sl],
                        start=False,
                        stop=(jp == npair - 1),
                        skip_group_check=True,
                    )
                    # softmax denominator tree
                    pa = pairp.tile([128, sl], BF16, name=f"pa{h}_{jp}", tag="pa")
                    nc.vector.tensor_add(pa, pr[:, 0:sl], pr[:, sl : 2 * sl])
                    pairs.append(pa)
                    if len(pairs) == 2:
                        q2 = pairp.tile([128, sl], BF16, name=f"q2{h}_{jp}", tag="q2")
                        nc.vector.tensor_add(q2, pairs[0], pairs[1])
                        pairs = []
                        if nq2 == 0:
                            nc.vector.tensor_copy(acc, q2)
                        else:
                            nc.vector.tensor_add(acc, acc, q2)
                        nq2 += 1
                if pairs:  # odd number of pairs (small test sizes)
                    if nq2 == 0:
                        nc.vector.tensor_copy(acc, pairs[0])
                    else:
                        nc.vector.tensor_add(acc, acc, pairs[0])
                    pairs = []
                ssum = ps_small.tile([1, sl], F32, name=f"ssum{h}", tag="ssq")
                nc.tensor.matmul(ssum, ones_sb, acc, start=True, stop=True)
                rs = stp.tile([1, sl], F32, name=f"rs{h}", tag="rsq")
                nc.vector.reciprocal(rs, ssum)
                rsb = rbp.tile([128, sl], F32, name=f"rsb{h}", tag="rsqb")
                nc.gpsimd.partition_broadcast(rsb, rs)
                nc.vector.tensor_mul(otn[:, sl * h : sl * (h + 1)], outp, rsb)

        # ---------------- phase 5: output projection ----------------
        with tc.tile_pool(name="wo", bufs=4) as wop, tc.tile_pool(
            name="ps_o", bufs=4, space="PSUM"
        ) as ps_o, tc.tile_pool(name="osb", bufs=2) as osb:
            for ds_ in range(D // 512):
                pso = [
                    ps_o.tile([128, 512], F32, name=f"pso{ds_}_{tt}", tag="pso")
                    for tt in range(nt)
                ]
                for hk in range(H):
                    wo_t = wop.tile([128, 512], BF16, name=f"wo{ds_}_{hk}", tag="wo")
                    nc.gpsimd.dma_start(
                        out=wo_t,
                        in_=wo_ap[128 * hk : 128 * (hk + 1), 512 * ds_ : 512 * (ds_ + 1)],
                    )
                    for tt in range(nt):
                        nc.tensor.matmul(
                            pso[tt],
                            otn[:, sl * hk + 128 * tt : sl * hk + 128 * (tt + 1)],
                            wo_t,
                            start=(hk == 0),
                            stop=(hk == H - 1),
                        )
                for tt in range(nt):
                    ob = osb.tile([128, 512], F32, name=f"ob{ds_}_{tt}", tag="ob")
                    nc.scalar.copy(ob, pso[tt])
                    nc.sync.dma_start(
                        out=out_ap[
                            128 * tt : 128 * (tt + 1), 512 * ds_ : 512 * (ds_ + 1)
                        ],
                        in_=ob,
                    )


def build(s_total=S, debug=False):
    """Build + compile the SPMD graph. Returns (nc, param name map)."""
    sl = s_total // NCORES
    nt = sl // 128
    kvper = KVH * 128 * sl + nt * 128 * (KVH * HD)
    nc = bacc.Bacc(
        "TRN2",
        target_bir_lowering=False,
        debug=debug,
        num_devices=NCORES,
    )
    prm = {}
    prm["x"] = nc.dram_tensor("x", [sl, D], F32, kind="ExternalInput")
    prm["wq"] = nc.dram_tensor("wq", [D, H * HD], F32, kind="ExternalInput")
    prm["wk"] = nc.dram_tensor("wk", [D, KVH * HD], F32, kind="ExternalInput")
    prm["wv"] = nc.dram_tensor("wv", [D, KVH * HD], F32, kind="ExternalInput")
    prm["wo"] = nc.dram_tensor("wo", [H * HD, D], F32, kind="ExternalInput")
    prm["costw_q"] = nc.dram_tensor("costw_q", [HD, sl], F32, kind="ExternalInput")
    prm["sintw_q"] = nc.dram_tensor("sintw_q", [HD, sl], F32, kind="ExternalInput")
    prm["costw_k"] = nc.dram_tensor("costw_k", [HD, sl], F32, kind="ExternalInput")
    prm["sintw_k"] = nc.dram_tensor("sintw_k", [HD, sl], F32, kind="ExternalInput")
    prm["out"] = nc.dram_tensor("out", [sl, D], F32, kind="ExternalOutput")
    prm["kv_in"] = nc.dram_tensor("kv_in", [kvper], BF16)
    prm["kv_out"] = nc.dram_tensor("kv_out", [NCORES * kvper], BF16, addr_space="Shared")
    rot = _rotmat_np()
    import ml_dtypes

    prm["rotmat"] = nc.inline_tensor(rot.astype(ml_dtypes.bfloat16), name="rotmat")

    with tile.TileContext(nc) as tc:
        _emit(tc, nc, prm, sl)
    nc.compile()
    return nc


def make_in_maps(inputs, s_total=S):
    """Shard the full inputs into the 8 per-core input maps (host-side numpy)."""
    sl = s_total // NCORES
    x = np.asarray(inputs["x"], dtype=np.float32).reshape(s_total, D)
    wq = np.asarray(inputs["Wq"], dtype=np.float32)
    wk = np.asarray(inputs["Wk"], dtype=np.float32)
    wv = np.asarray(inputs["Wv"], dtype=np.float32)
    wo = np.asarray(inputs["Wo"], dtype=np.float32)
    qw = np.asarray(inputs["q_norm_w"], dtype=np.float32)
    kw = np.asarray(inputs["k_norm_w"], dtype=np.float32)
    cosc = np.asarray(inputs["cos_cache"], dtype=np.float32)
    sinc = np.asarray(inputs["sin_cache"], dtype=np.float32)

    # cos/sin caches in transposed, duplicated layout with norm weights folded:
    #   costw[d, t] = cos[t, d % 64] * w[d]
    #   sintw[d, t] = sin[t, d % 64] * w[(d + 64) % 128]
    didx = np.arange(HD)
    wshift = np.roll if False else None  # noqa: F841  (clarity only)
    in_maps = []
    for i in range(NCORES):
        t0 = i * sl
        cos_t = cosc[t0 : t0 + sl, :].T  # [64, sl]
        sin_t = sinc[t0 : t0 + sl, :].T
        cos_full = cos_t[didx % 64, :]  # [128, sl]
        sin_full = sin_t[didx % 64, :]
        costw_q = (cos_full * qw[:, None]).astype(np.float32)
        sintw_q = (sin_full * qw[(didx + 64) % HD, None]).astype(np.float32)
        costw_k = (cos_full * kw[:, None]).astype(np.float32)
        sintw_k = (sin_full * kw[(didx + 64) % HD, None]).astype(np.float32)
        in_maps.append(
            {
                "x": np.ascontiguousarray(x[t0 : t0 + sl, :]),
                "wq": wq,
                "wk": wk,
                "wv": wv,
                "wo": wo,
                "costw_q": np.ascontiguousarray(costw_q),
                "sintw_q": np.ascontiguousarray(sintw_q),
                "costw_k": np.ascontiguousarray(costw_k),
                "sintw_k": np.ascontiguousarray(sintw_k),
            }
        )
    return in_maps


_NC_CACHE = {}
LAST_EXEC_TIME_NS = None
LAST_RESULTS = None


def _get_nc(s_total=S):
    if s_total not in _NC_CACHE:
        _NC_CACHE[s_total] = build(s_total)
    return _NC_CACHE[s_total]


def kernel(**inputs):
    global LAST_EXEC_TIME_NS, LAST_RESULTS
    nc = _get_nc(S)
    in_maps = make_in_maps(inputs, S)
    kw = {}
    if os.environ.get("ATTN_TRACE", "0") == "1":
        try:
            import antenv.axon_hooks  # noqa: F401

            kw["trace"] = True
            td = os.environ.get("ATTN_TRACE_DIR")
            if td:
                os.makedirs(td, exist_ok=True)
                kw["tmpdir"] = td
        except ImportError:
            pass
    res = run_bass_kernel_spmd(nc, in_maps, list(range(NCORES)), **kw)
    LAST_EXEC_TIME_NS = res.exec_time_ns
    LAST_RESULTS = res
    sl = S // NCORES
    out = np.concatenate(
        [np.asarray(res.results[i]["out"]).reshape(sl, D) for i in range(NCORES)],
        axis=0,
    )
    return out.reshape(B, S, D).astype(np.float32)


# revision 18
# speedup vs baseline: 7486.7002x; 7486.7002x over previous
"""Distributed Bass kernel for nn_Attention_12635793785074 (8 TRN2 NeuronCores).

Sharding: sequence-parallel. Core i owns query tokens [i*SL, (i+1)*SL).
Each core computes Q/K/V projections for its own tokens (all heads),
AllGathers the (normalized, roped, bf16) K^T and V across the 8 cores,
runs full attention for its queries over all 4096 keys, and applies the
output projection for its rows. The full output is the concatenation of
per-core outputs (no output collective needed).
"""

import os
import sys

for _p in (
    "/opt/trn_rl_repo",
    "/opt/pypackages",
    "/root/.axon_site/_ro/trn_rl_repo",
    "/root/.axon_site/_ro/pypackages",
):
    if os.path.isdir(_p) and _p not in sys.path:
        sys.path.append(_p)

import numpy as np  # noqa: E402

import concourse.bass as bass  # noqa: E402
import concourse.mybir as mybir  # noqa: E402
import concourse.tile as tile  # noqa: E402
from concourse import bacc  # noqa: E402
from concourse.bass_utils import run_bass_kernel_spmd  # noqa: E402

F32 = mybir.dt.float32
F32R = mybir.dt.float32r
BF16 = mybir.dt.bfloat16

B, S, D = 1, 4096, 2048
H, KVH, HD = 16, 4, 128
G = H // KVH  # 4 query heads per kv head
NCORES = 8
EPS = 1e-6
ISQ = 1.0 / float(np.sqrt(HD))  # attention scale, folded into exp()


def _rotmat_np():
    """P^T for rot(v) = P @ v with rot(v)[d] = -v[d+64] (d<64), v[d-64] (d>=64)."""
    P = np.zeros((HD, HD), dtype=np.float32)
    for dd in range(64):
        P[dd, dd + 64] = -1.0
        P[dd + 64, dd] = 1.0
    return P.T.copy()


def _emit(tc, nc, prm, sl, pfx=""):
    """Emit the per-core program. sl = local sequence length (tokens per core)."""
    from contextlib import ExitStack

    nt = sl // 128          # local token tiles
    ltn = (sl * NCORES) // 128  # total l (key) tiles
    ftn = D // 128          # feature tiles of the contraction dim

    x_ap = prm["x"].ap()
    wq_ap = prm["wq"].ap()
    wk_ap = prm["wk"].ap()
    wv_ap = prm["wv"].ap()
    wo_ap = prm["wo"].ap()
    out_ap = prm["out"].ap()
    kv_in = prm["kv_in"]
    kv_out = prm["kv_out"]
    kvper = KVH * 128 * sl + nt * 128 * (KVH * HD)  # elements per core in kv bounce

    def kv_in_k(kh):
        off = kh * 128 * sl
        return kv_in.ap()[off : off + 128 * sl].rearrange("(p t) -> p t", t=sl)

    def kv_in_v(tt):
        off = KVH * 128 * sl + tt * 128 * (KVH * HD)
        return kv_in.ap()[off : off + 128 * (KVH * HD)].rearrange(
            "(p c) -> p c", c=KVH * HD
        )

    def kv_out_k(s_, kh):
        off = kvper * s_ + kh * 128 * sl
        return kv_out.ap()[off : off + 128 * sl].rearrange("(p t) -> p t", t=sl)

    def kv_out_v(s_, tt):
        off = kvper * s_ + KVH * 128 * sl + tt * 128 * (KVH * HD)
        return kv_out.ap()[off : off + 128 * (KVH * HD)].rearrange(
            "(p c) -> p c", c=KVH * HD
        )

    es = ExitStack()
    with es:
        # ---------------- constants ----------------
        consts = es.enter_context(tc.tile_pool(name=pfx + "consts", bufs=1))
        rotmat_sb = consts.tile([HD, HD], BF16, name=pfx + "rotm")
        nc.sync.dma_start(out=rotmat_sb, in_=prm["rotmat"].ap())
        ones_f = consts.tile([128, 1], F32, name=pfx + "onesf")
        nc.vector.memset(ones_f, 1.0)
        ones_sb = consts.tile([128, 1], F32R, name=pfx + "ones")
        nc.vector.tensor_copy(ones_sb, ones_f)
        costwq_sb = consts.tile([128, sl], F32, name=pfx + "cwq")
        sintwq_sb = consts.tile([128, sl], F32, name=pfx + "swq")
        costwk_sb = consts.tile([128, sl], F32, name=pfx + "cwk")
        sintwk_sb = consts.tile([128, sl], F32, name=pfx + "swk")
        nc.sync.dma_start(out=costwq_sb, in_=prm["costw_q"].ap())
        nc.sync.dma_start(out=sintwq_sb, in_=prm["sintw_q"].ap())
        nc.sync.dma_start(out=costwk_sb, in_=prm["costw_k"].ap())
        nc.sync.dma_start(out=sintwk_sb, in_=prm["sintw_k"].ap())
        id_bf = consts.tile([128, 128], BF16, name=pfx + "idbf")
        from concourse.masks import make_identity

        make_identity(nc, id_bf)
        zbias = consts.tile([128, 1], F32, name=pfx + "zb")
        nc.vector.memset(zbias, 0.0)
        ebias = consts.tile([128, 1], F32, name=pfx + "eb")
        nc.vector.memset(ebias, EPS)

        # persistent big tensors
        bigs = es.enter_context(tc.tile_pool(name=pfx + "bigs", bufs=1))
        xT = bigs.tile([128, ftn * sl], BF16, name=pfx + "xT")        # x^T, [feat, tok]
        qt = bigs.tile([128, H * sl], BF16, name=pfx + "qt")          # roped Q^T per head
        kt = bigs.tile([128, KVH * ltn * 128], BF16, name=pfx + "kt")  # gathered K^T
        vsb = bigs.tile([128, ltn * (KVH * HD)], BF16, name=pfx + "vsb")  # gathered V
        otn = bigs.tile([128, H * sl], BF16, name=pfx + "otn")        # normalized attn out^T

        # ---------------- phase 1: x -> x^T (bf16) ----------------
        with tc.tile_pool(name=pfx + "xload", bufs=nt) as xload, tc.tile_pool(
            name=pfx + "ps_x", bufs=2, space="PSUM"
        ) as ps_x:
            xbf = []
            for tt in range(nt):
                xb = xload.tile([128, D], BF16, name=pfx + f"xb{tt}", tag="xb")
                nc.gpsimd.dma_start(out=xb, in_=x_ap[128 * tt : 128 * (tt + 1), :])
                xbf.append(xb)
            for ff in range(ftn):
                pst = ps_x.tile([128, nt * 128], BF16, name=pfx + "pst", tag="pst")
                for tt in range(nt):
                    nc.tensor.transpose(
                        pst[:, 128 * tt : 128 * (tt + 1)],
                        xbf[tt][:, 128 * ff : 128 * (ff + 1)],
                        id_bf,
                    )
                nc.vector.tensor_copy(xT[:, sl * ff : sl * ff + nt * 128], pst)

        # shared pools for projection-phase norm/rope pipeline
        rbp = es.enter_context(tc.tile_pool(name=pfx + "rbp", bufs=3))
        stp = es.enter_context(tc.tile_pool(name=pfx + "stp", bufs=4))
        ps_small = es.enter_context(tc.tile_pool(name=pfx + "ps_small", bufs=2, space="PSUM"))
        proj_es = ExitStack()
        pp = proj_es.enter_context(tc.tile_pool(name=pfx + "prep", bufs=3))
        ps_rot = proj_es.enter_context(tc.tile_pool(name=pfx + "ps_rot", bufs=2, space="PSUM"))

        def norm_rope(psrc, cosw, sinw, dst_ap, hname):
            """RMS-norm + RoPE of one head's [HD, sl] transposed proj output."""
            raw = pp.tile([128, sl], F32, name=pfx + f"raw{hname}", tag="raw")
            nc.scalar.copy(raw, psrc)  # PSUM -> SBUF
            sq = pp.tile([128, sl], F32R, name=pfx + f"sq{hname}", tag="sq")
            nc.vector.tensor_mul(sq, raw, raw)
            ssq = ps_small.tile([1, sl], F32, name=pfx + f"ssq{hname}", tag="ssq")
            nc.tensor.matmul(ssq, ones_sb, sq, start=True, stop=True)
            sqs = stp.tile([1, sl], F32, name=pfx + f"sqs{hname}", tag="sqs")
            nc.scalar.activation(
                sqs,
                ssq,
                mybir.ActivationFunctionType.Sqrt,
                bias=ebias[0:1, :],
                scale=1.0 / HD,
            )
            rsq = stp.tile([1, sl], F32, name=pfx + f"rsq{hname}", tag="rsq")
            nc.vector.reciprocal(rsq, sqs)
            rsqb = rbp.tile([128, sl], F32, name=pfx + f"rsqb{hname}", tag="rsqb")
            nc.gpsimd.partition_broadcast(rsqb, rsq)
            qn = pp.tile([128, sl], BF16, name=pfx + f"qn{hname}", tag="qn")
            nc.vector.tensor_mul(qn, raw, rsqb)
            rot = ps_rot.tile([128, sl], F32, name=pfx + f"rot{hname}", tag="rot")
            nc.tensor.matmul(rot, rotmat_sb, qn, start=True, stop=True)
            t1 = pp.tile([128, sl], F32, name=pfx + f"t1{hname}", tag="t1")
            nc.vector.tensor_mul(t1, qn, cosw)
            t2 = pp.tile([128, sl], F32, name=pfx + f"t2{hname}", tag="t2")
            nc.vector.tensor_mul(t2, rot, sinw)
            nc.vector.tensor_add(dst_ap, t1, t2)

        # ---------------- phase 2: K^T, V, collective ----------------
        klocal = proj_es.enter_context(tc.tile_pool(name=pfx + "klocal", bufs=KVH + nt))
        with tc.tile_pool(name=pfx + "wkv", bufs=3) as wkv, tc.tile_pool(
            name="ps_proj", bufs=4, space="PSUM"
        ) as ps_proj:
            psk = [
                ps_proj.tile([128, sl], F32, name=pfx + f"psk{kh}", tag="projps")
                for kh in range(KVH)
            ]
            for ff in range(ftn):
                wk_t = wkv.tile([128, KVH * HD], BF16, name=pfx + f"wk{ff}", tag="wk")
                nc.gpsimd.dma_start(out=wk_t, in_=wk_ap[128 * ff : 128 * (ff + 1), :])
                for kh in range(KVH):
                    nc.tensor.matmul(
                        psk[kh],
                        wk_t[:, 128 * kh : 128 * (kh + 1)],
                        xT[:, sl * ff : sl * (ff + 1)],
                        start=(ff == 0),
                        stop=(ff == ftn - 1),
                    )
            for kh in range(KVH):
                kf = klocal.tile([128, sl], BF16, name=pfx + f"kf{kh}", tag="kf")
                norm_rope(psk[kh], costwk_sb, sintwk_sb, kf, f"k{kh}")
                nc.sync.dma_start(out=kv_in_k(kh), in_=kf)

            psv = [
                ps_proj.tile([128, KVH * HD], F32, name=pfx + f"psv{tt}", tag="projps")
                for tt in range(nt)
            ]
            for ff in range(ftn):
                wv_t = wkv.tile([128, KVH * HD], BF16, name=pfx + f"wv{ff}", tag="wk")
                nc.gpsimd.dma_start(out=wv_t, in_=wv_ap[128 * ff : 128 * (ff + 1), :])
                for tt in range(nt):
                    nc.tensor.matmul(
                        psv[tt],
                        xT[:, sl * ff + 128 * tt : sl * ff + 128 * (tt + 1)],
                        wv_t,
                        start=(ff == 0),
                        stop=(ff == ftn - 1),
                    )
            for tt in range(nt):
                vf = klocal.tile([128, KVH * HD], BF16, name=pfx + f"vf{tt}", tag="kf")
                nc.vector.tensor_copy(vf, psv[tt])
                nc.sync.dma_start(out=kv_in_v(tt), in_=vf)

        nc.gpsimd.collective_compute(
            "AllGather",
            mybir.AluOpType.bypass,
            replica_groups=[list(range(NCORES))],
            ins=[kv_in.ap().opt()],
            outs=[kv_out.ap().opt()],
        )
        # scatter gathered kv into SBUF layouts
        for kh in range(KVH):
            for s_ in range(NCORES):
                nc.sync.dma_start(
                    out=kt[:, ltn * 128 * kh + sl * s_ : ltn * 128 * kh + sl * s_ + sl],
                    in_=kv_out_k(s_, kh),
                )
        for s_ in range(NCORES):
            for tt in range(nt):
                c0 = (KVH * HD) * (nt * s_ + tt)
                nc.sync.dma_start(
                    out=vsb[:, c0 : c0 + KVH * HD],
                    in_=kv_out_v(s_, tt),
                )

        # ---------------- phase 3: Q ----------------
        with tc.tile_pool(name=pfx + "wq", bufs=3) as wqp, tc.tile_pool(
            name="ps_projq", bufs=4, space="PSUM"
        ) as ps_projq:
            for hg in range(H // 4):
                psq = [
                    ps_projq.tile([128, sl], F32, name=pfx + f"psq{hg}_{hh}", tag="projqps")
                    for hh in range(4)
                ]
                for ff in range(ftn):
                    wq_t = wqp.tile([128, 512], BF16, name=pfx + f"wq{hg}_{ff}", tag="wq")
                    nc.gpsimd.dma_start(
                        out=wq_t,
                        in_=wq_ap[128 * ff : 128 * (ff + 1), 512 * hg : 512 * (hg + 1)],
                    )
                    for hh in range(4):
                        nc.tensor.matmul(
                            psq[hh],
                            wq_t[:, 128 * hh : 128 * (hh + 1)],
                            xT[:, sl * ff : sl * (ff + 1)],
                            start=(ff == 0),
                            stop=(ff == ftn - 1),
                        )
                for hh in range(4):
                    h = 4 * hg + hh
                    norm_rope(
                        psq[hh],
                        costwq_sb,
                        sintwq_sb,
                        qt[:, sl * h : sl * (h + 1)],
                        f"q{h}",
                    )

        proj_es.close()  # release proj-phase PSUM banks before attention

        # ---------------- phase 4: attention ----------------
        with tc.tile_pool(name=pfx + "ps_sc", bufs=2, space="PSUM") as ps_sc, tc.tile_pool(
            name="ps_out", bufs=2, space="PSUM"
        ) as ps_out, tc.tile_pool(name=pfx + "prpool", bufs=4) as prpool, tc.tile_pool(
            name="pairp", bufs=6
        ) as pairp, tc.tile_pool(name=pfx + "accp", bufs=2) as accp:
            npair = ltn // 2
            for h in range(H):
                kh = h // G
                qh = qt[:, sl * h : sl * (h + 1)]
                outp = ps_out.tile([128, sl], F32, name=pfx + f"outp{h}", tag="outp")
                acc = accp.tile([128, sl], F32R, name=pfx + f"acc{h}", tag="acc")
                pairs = []
                nq2 = 0
                for jp in range(npair):
                    j0, j1 = 2 * jp, 2 * jp + 1
                    sc = ps_sc.tile([128, 2 * sl], F32, name=pfx + f"sc{h}_{jp}", tag="sc")
                    nc.tensor.matmul(
                        sc[:, 0:sl],
                        kt[:, ltn * 128 * kh + 128 * j0 : ltn * 128 * kh + 128 * (j0 + 1)],
                        qh,
                        start=True,
                        stop=True,
                    )
                    nc.tensor.matmul(
                        sc[:, sl : 2 * sl],
                        kt[:, ltn * 128 * kh + 128 * j1 : ltn * 128 * kh + 128 * (j1 + 1)],
                        qh,
                        start=True,
                        stop=True,
                    )
                    pr = prpool.tile([128, 2 * sl], BF16, name=pfx + f"pr{h}_{jp}", tag="pr")
                    nc.scalar.activation(
                        pr,
                        sc,
                        mybir.ActivationFunctionType.Exp,
                        bias=zbias[:, :],
                        scale=ISQ,
                    )
                    # PV accumulate
                    nc.tensor.matmul(
                        outp,
                        vsb[:, (KVH * HD) * j0 + HD * kh : (KVH * HD) * j0 + HD * (kh + 1)],
                        pr[:, 0:sl],
                        start=(jp == 0),
                        stop=False,
                        skip_group_check=True,
                    )
                    nc.tensor.matmul(
                        outp,
                        vsb[:, (KVH * HD) * j1 + HD * kh : (KVH * HD) * j1 + HD * (kh + 1)],
                        pr[:, sl : 2 * sl],
                        start=False,
                        stop=(jp == npair - 1),
                        skip_group_check=True,
                    )
                    # softmax denominator tree
                    pa = pairp.tile([128, sl], BF16, name=pfx + f"pa{h}_{jp}", tag="pa")
                    nc.vector.tensor_add(pa, pr[:, 0:sl], pr[:, sl : 2 * sl])
                    pairs.append(pa)
                    if len(pairs) == 2:
                        q2 = pairp.tile([128, sl], BF16, name=pfx + f"q2{h}_{jp}", tag="q2")
                        nc.vector.tensor_add(q2, pairs[0], pairs[1])
                        pairs = []
                        if nq2 == 0:
                            nc.vector.tensor_copy(acc, q2)
                        else:
                            nc.vector.tensor_add(acc, acc, q2)
                        nq2 += 1
                if pairs:  # odd number of pairs (small test sizes)
                    if nq2 == 0:
                        nc.vector.tensor_copy(acc, pairs[0])
                    else:
                        nc.vector.tensor_add(acc, acc, pairs[0])
                    pairs = []
                ssum = ps_small.tile([1, sl], F32, name=pfx + f"ssum{h}", tag="ssq")
                nc.tensor.matmul(ssum, ones_sb, acc, start=True, stop=True)
                rs = stp.tile([1, sl], F32, name=pfx + f"rs{h}", tag="rsq")
                nc.vector.reciprocal(rs, ssum)
                rsb = rbp.tile([128, sl], F32, name=pfx + f"rsb{h}", tag="rsqb")
                nc.gpsimd.partition_broadcast(rsb, rs)
                nc.vector.tensor_mul(otn[:, sl * h : sl * (h + 1)], outp, rsb)

        # ---------------- phase 5: output projection ----------------
        with tc.tile_pool(name=pfx + "wo", bufs=4) as wop, tc.tile_pool(
            name="ps_o", bufs=4, space="PSUM"
        ) as ps_o, tc.tile_pool(name=pfx + "osb", bufs=2) as osb:
            for ds_ in range(D // 512):
                pso = [
                    ps_o.tile([128, 512], F32, name=pfx + f"pso{ds_}_{tt}", tag="pso")
                    for tt in range(nt)
                ]
                for hk in range(H):
                    wo_t = wop.tile([128, 512], BF16, name=pfx + f"wo{ds_}_{hk}", tag="wo")
                    nc.gpsimd.dma_start(
                        out=wo_t,
                        in_=wo_ap[128 * hk : 128 * (hk + 1), 512 * ds_ : 512 * (ds_ + 1)],
                    )
                    for tt in range(nt):
                        nc.tensor.matmul(
                            pso[tt],
                            otn[:, sl * hk + 128 * tt : sl * hk + 128 * (tt + 1)],
                            wo_t,
                            start=(hk == 0),
                            stop=(hk == H - 1),
                        )
                for tt in range(nt):
                    ob = osb.tile([128, 512], F32, name=pfx + f"ob{ds_}_{tt}", tag="ob")
                    nc.scalar.copy(ob, pso[tt])
                    nc.sync.dma_start(
                        out=out_ap[
                            128 * tt : 128 * (tt + 1), 512 * ds_ : 512 * (ds_ + 1)
                        ],
                        in_=ob,
                    )


def build(s_total=S, debug=False, reps=1):
    """Build + compile the SPMD graph. Returns (nc, param name map)."""
    sl = s_total // NCORES
    nt = sl // 128
    kvper = KVH * 128 * sl + nt * 128 * (KVH * HD)
    nc = bacc.Bacc(
        "TRN2",
        target_bir_lowering=False,
        debug=debug,
        num_devices=NCORES,
    )
    prm = {}
    prm["x"] = nc.dram_tensor("x", [sl, D], F32, kind="ExternalInput")
    prm["wq"] = nc.dram_tensor("wq", [D, H * HD], F32, kind="ExternalInput")
    prm["wk"] = nc.dram_tensor("wk", [D, KVH * HD], F32, kind="ExternalInput")
    prm["wv"] = nc.dram_tensor("wv", [D, KVH * HD], F32, kind="ExternalInput")
    prm["wo"] = nc.dram_tensor("wo", [H * HD, D], F32, kind="ExternalInput")
    prm["costw_q"] = nc.dram_tensor("costw_q", [HD, sl], F32, kind="ExternalInput")
    prm["sintw_q"] = nc.dram_tensor("sintw_q", [HD, sl], F32, kind="ExternalInput")
    prm["costw_k"] = nc.dram_tensor("costw_k", [HD, sl], F32, kind="ExternalInput")
    prm["sintw_k"] = nc.dram_tensor("sintw_k", [HD, sl], F32, kind="ExternalInput")
    prm["out"] = nc.dram_tensor("out", [sl, D], F32, kind="ExternalOutput")
    prm["kv_in"] = nc.dram_tensor("kv_in", [kvper], BF16)
    prm["kv_out"] = nc.dram_tensor("kv_out", [NCORES * kvper], BF16, addr_space="Shared")
    rot = _rotmat_np()
    import ml_dtypes

    prm["rotmat"] = nc.inline_tensor(rot.astype(ml_dtypes.bfloat16), name="rotmat")

    with tile.TileContext(nc) as tc:
        for rep in range(reps):
            _emit(tc, nc, prm, sl, pfx=f"r{rep}_" if reps > 1 else "")
    nc.compile()
    return nc


def make_in_maps(inputs, s_total=S):
    """Shard the full inputs into the 8 per-core input maps (host-side numpy)."""
    sl = s_total // NCORES
    x = np.asarray(inputs["x"], dtype=np.float32).reshape(s_total, D)
    wq = np.asarray(inputs["Wq"], dtype=np.float32)
    wk = np.asarray(inputs["Wk"], dtype=np.float32)
    wv = np.asarray(inputs["Wv"], dtype=np.float32)
    wo = np.asarray(inputs["Wo"], dtype=np.float32)
    qw = np.asarray(inputs["q_norm_w"], dtype=np.float32)
    kw = np.asarray(inputs["k_norm_w"], dtype=np.float32)
    cosc = np.asarray(inputs["cos_cache"], dtype=np.float32)
    sinc = np.asarray(inputs["sin_cache"], dtype=np.float32)

    # cos/sin caches in transposed, duplicated layout with norm weights folded:
    #   costw[d, t] = cos[t, d % 64] * w[d]
    #   sintw[d, t] = sin[t, d % 64] * w[(d + 64) % 128]
    didx = np.arange(HD)
    wshift = np.roll if False else None  # noqa: F841  (clarity only)
    in_maps = []
    for i in range(NCORES):
        t0 = i * sl
        cos_t = cosc[t0 : t0 + sl, :].T  # [64, sl]
        sin_t = sinc[t0 : t0 + sl, :].T
        cos_full = cos_t[didx % 64, :]  # [128, sl]
        sin_full = sin_t[didx % 64, :]
        costw_q = (cos_full * qw[:, None]).astype(np.float32)
        sintw_q = (sin_full * qw[(didx + 64) % HD, None]).astype(np.float32)
        costw_k = (cos_full * kw[:, None]).astype(np.float32)
        sintw_k = (sin_full * kw[(didx + 64) % HD, None]).astype(np.float32)
        in_maps.append(
            {
                "x": np.ascontiguousarray(x[t0 : t0 + sl, :]),
                "wq": wq,
                "wk": wk,
                "wv": wv,
                "wo": wo,
                "costw_q": np.ascontiguousarray(costw_q),
                "sintw_q": np.ascontiguousarray(sintw_q),
                "costw_k": np.ascontiguousarray(costw_k),
                "sintw_k": np.ascontiguousarray(sintw_k),
            }
        )
    return in_maps


_NC_CACHE = {}
LAST_EXEC_TIME_NS = None
LAST_RESULTS = None


def _get_nc(s_total=S):
    if s_total not in _NC_CACHE:
        _NC_CACHE[s_total] = build(s_total)
    return _NC_CACHE[s_total]


def kernel(**inputs):
    global LAST_EXEC_TIME_NS, LAST_RESULTS
    nc = _get_nc(S)
    in_maps = make_in_maps(inputs, S)
    kw = {}
    if os.environ.get("ATTN_TRACE", "0") == "1":
        try:
            import antenv.axon_hooks  # noqa: F401

            kw["trace"] = True
            td = os.environ.get("ATTN_TRACE_DIR")
            if td:
                os.makedirs(td, exist_ok=True)
                kw["tmpdir"] = td
        except ImportError:
            pass
    res = run_bass_kernel_spmd(nc, in_maps, list(range(NCORES)), **kw)
    LAST_EXEC_TIME_NS = res.exec_time_ns
    LAST_RESULTS = res
    sl = S // NCORES
    out = np.concatenate(
        [np.asarray(res.results[i]["out"]).reshape(sl, D) for i in range(NCORES)],
        axis=0,
    )
    return out.reshape(B, S, D).astype(np.float32)
